# revision 15
# baseline (speedup 1.0000x reference)
"""Trainium2 Bass kernel for nn_Align: batched quaternion->rotmat + rigid transform.

reference math (per structure j of 64):
    q = (1, b, c, d) / sqrt(s),  s = 1 + b^2 + c^2 + d^2
    R = rotmat(q)                       # 3x3
    out[j] = pred[j] @ R + t[j]         # [91,3] @ [3,3] + [3]

Sharding: data-parallel over the 8 NeuronCores, 8 structures per core.

Per-core layout: partitions = (structure j:8, out-channel n:3, point-group
g:5) = 120, with 19 points per group (91 points padded to 95; the pad is
sliced off host-side).  Every rotation-matrix entry has the form
    R[k,n] = (x*y + z) * (2/s) - w
with (x,y,z,w) in {1,b,c,d,0,+-...}, so column n of R is computed per
partition from host-packed operand vectors X,Y,Z,W via
    C = (X*Y + Z) * (2/s) - W            # C = (R[0,n], R[1,n], R[2,n])
and s comes from one fused tensor_tensor_reduce (products + scaled sum).
The transform is then only 3 multiply-add ops over the 19-point free dim:
    out_n = ((x0*C0 + t_n) + x1*C1) + x2*C2
Output is written channel-planar ([3,8,95] DRAM) so each partition's row is
contiguous; the host interleaves channels during the unshard gather.

Two parallel input DMAs: the small params tensor goes through the SP queue
(fast DGE path -- it gates the compute chain) while the 3x-replicated
coordinate tensor rides the otherwise-idle Activation-engine queue.

Raw Bass (no Tile: this walrus build encodes at most one sync-wait per
compute instruction).  Every DVE RAW dep is semaphore-synced (streaming
same-engine RAW is not safe on HW), and the kernel clears its semaphores
then all-engine-barriers before use (sems persist across NEFF runs).
"""

import numpy as np

NCORES = 8
J = 8          # structures per core
N = 3          # output channels (partition dim)
G = 5          # point groups per (structure, channel)
Q = 19         # points per group (G*Q = 95 >= 91)
NPTS = 91
PADPTS = G * Q  # 95
PARTS = J * N * G  # 120 partitions

# params row: [A(10): 1 b c d x0 x1 x2 x0 x1 x2 |
#              B(10): .5 .5b .5c .5d y0 y1 y2 -y0 -y1 -y2 |
#              Z(3) | W(3) | t(1)] = 27 floats
# One fused (A*B, accum) op then yields the column products AND s/2: the
# +-x*y accumulator slots cancel, leaving 0.5*(1+b^2+c^2+d^2).
PA, PBv, PZ, PW, PT, PLEN = 0, 10, 20, 23, 26, 27

_cache = {}


def _build_nc():
    import concourse.bass as bass
    import concourse.mybir as mybir

    f32 = mybir.dt.float32
    Alu = mybir.AluOpType

    nc = bass.Bass()
    pk_d = nc.dram_tensor("pk", [PARTS, PLEN], f32, kind="ExternalInput")
    xt_d = nc.dram_tensor("xt", [PARTS, 3 * Q], f32, kind="ExternalInput")
    # channel-planar output: [j, n, 95]; host transposes/slices to [j,91,n]
    o3 = nc.dram_tensor("o3", [J, N, PADPTS], f32, kind="ExternalOutput")

    with (
        nc.sbuf_tensor([PARTS, PLEN], f32) as PK_t,
        nc.sbuf_tensor([PARTS, 3 * Q], f32) as XT_t,
        nc.sbuf_tensor([PARTS, 10], f32) as PR_t,
        nc.sbuf_tensor([PARTS, 1], f32) as S2_t,
        nc.sbuf_tensor([PARTS, 1], f32) as INV_t,
        nc.sbuf_tensor([PARTS, 3], f32) as NU_t,
        nc.sbuf_tensor([PARTS, 3], f32) as C_t,
        nc.sbuf_tensor([PARTS, Q], f32) as A1_t,
        nc.sbuf_tensor([PARTS, Q], f32) as A2_t,
        nc.sbuf_tensor([PARTS, Q], f32) as O_t,
        nc.semaphore("p_sem") as p_sem,
        nc.semaphore("x_sem") as x_sem,
        nc.semaphore("v_sem") as v_sem,
        nc.semaphore("o_sem") as o_sem,
        nc.Block() as block,
    ):
        PK = PK_t[:, :]
        XT = XT_t[:, :]
        A1 = A1_t[:, :]
        A2 = A2_t[:, :]
        O = O_t[:, :]

        # Stale-semaphore protocol: semaphores are NOT reset between NEFF
        # executions, and waits here use absolute values, so every engine
        # CLEARS the sems it waits on before its first wait (same-engine
        # program order makes clear-before-wait unconditional -- no
        # cross-engine barrier needed; v_sem is double-cleared because both
        # DVE and SP wait on it, and a clear is idempotent).  Increment
        # races are margin-safe: a HWDGE completion inc lands >=1.5us after
        # its dma_start (descriptor gen + DGE delay + transfer + sem prop)
        # while all clears retire within the first few hundred ns, and DVE's
        # v_sem incs only begin after the params DMA completes.  gpsimd
        # drains stale foreign DMA state for the sems that have no DMA in
        # flight during the preamble (draining p/x would nuke the two input
        # DMAs' in-flight descriptors; their queues are made safe by the
        # previous run of this kernel having fully retired both input DMAs
        # before its exit barrier).
        assert x_sem.num == p_sem.num + 1, (p_sem.num, x_sem.num)
        assert v_sem.num == x_sem.num + 1 and o_sem.num == v_sem.num + 1

        @block.gpsimd
        def _(gpsimd):
            gpsimd.dma_reset(range(v_sem.num, o_sem.num + 1))

        @block.tensor
        def _(tensor):
            pass

        @block.scalar
        def _(scalar):
            # coordinate tensor rides the Activation HWDGE queue, parallel
            # with the params DMA on SP; Act has no sem waits, so it needs
            # no clears
            scalar.dma_start(out=XT, in_=xt_d[:, :]).then_inc(x_sem, 16)

        @block.sync
        def _(sync):
            sync.dma_start(out=PK, in_=pk_d[:, :]).then_inc(p_sem, 16)
            sync.sem_clear(range(v_sem.num, o_sem.num + 1))
            sync.wait_ge(v_sem, 7)
            sync.dma_start(
                out=o3[:, :, :].rearrange("j n (g q) -> (j n g) q", g=G),
                in_=O,
            ).then_inc(o_sem, 16)
            sync.wait_ge(o_sem, 16)

        @block.vector
        def _(vector):
            vector.sem_clear(range(p_sem.num, v_sem.num + 1))
            vector.wait_ge(p_sem, 16)

            # DVE streaming RAW is not safe without sem sync (HW-verified):
            # every op bumps v_sem; consumers wait on the cumulative count.
            def op(k, *args, **kw):
                return getattr(vector, k)(*args, **kw).then_inc(v_sem, 1)

            # ---- rotation column C = (X*Y + Z) * (2/s) - W ----
            # one fused op: PR = A*B elementwise (PR[4:7] = x*y column
            # products) and accum = sum(PR) = s/2 (the +-x*y slots cancel).
            # (tensor_tensor_reduce hits "ISA wrong length" in neuronxcc
            # codegen; scalar_tensor_tensor's accum_out compiles fine)
            op("scalar_tensor_tensor", out=PR_t[:, :],                   # 1
               in0=PK[:, PA:PA + 10], scalar=1.0, in1=PK[:, PBv:PBv + 10],
               op0=Alu.mult, op1=Alu.mult, accum_out=S2_t[:, :])
            vector.wait_ge(v_sem, 1)
            op("reciprocal", out=INV_t[:, :], in_=S2_t[:, :])            # 2  2/s
            op("tensor_tensor", out=NU_t[:, :], in0=PR_t[:, 4:7],        # 3
               in1=PK[:, PZ:PZ + 3], op=Alu.add)
            vector.wait_ge(v_sem, 3)
            op("scalar_tensor_tensor", out=C_t[:, :], in0=NU_t[:, :],    # 4
               scalar=INV_t[:, 0:1], in1=PK[:, PW:PW + 3],
               op0=Alu.mult, op1=Alu.subtract)

            # ---- transform: out_n = ((x0*C0 + t) + x1*C1) + x2*C2 ----
            vector.wait_ge(x_sem, 16)
            vector.wait_ge(v_sem, 4)
            op("tensor_scalar", out=A1, in0=XT[:, 0:Q],                  # 5
               scalar1=C_t[:, 0:1], scalar2=PK[:, PT:PT + 1],
               op0=Alu.mult, op1=Alu.add)
            vector.wait_ge(v_sem, 5)
            op("scalar_tensor_tensor", out=A2, in0=XT[:, Q:2 * Q],       # 6
               scalar=C_t[:, 1:2], in1=A1, op0=Alu.mult, op1=Alu.add)
            vector.wait_ge(v_sem, 6)
            op("scalar_tensor_tensor", out=O, in0=XT[:, 2 * Q:3 * Q],    # 7
               scalar=C_t[:, 2:3], in1=A2, op0=Alu.mult, op1=Alu.add)

    return nc


def get_nc():
    if "nc" not in _cache:
        _cache["nc"] = _build_nc()
    return _cache["nc"]


def shard_inputs(pred_coor, r_vector, t_vector):
    n_total = pred_coor.shape[0]
    b, c, d = r_vector[:, 0], r_vector[:, 1], r_vector[:, 2]
    one = np.ones_like(b)
    zero = np.zeros_like(b)

    # per-channel operand vectors: R[k,n] = (x*y+z)*(2/s) - w
    X = np.empty((n_total, N, 3), dtype=np.float32)
    Y = np.empty((n_total, N, 3), dtype=np.float32)
    Z = np.empty((n_total, N, 3), dtype=np.float32)
    W = np.empty((n_total, N, 3), dtype=np.float32)
    X[:, 0] = np.stack([b, b, b], -1)
    Y[:, 0] = np.stack([b, c, d], -1)
    Z[:, 0] = np.stack([one, d, -c], -1)
    W[:, 0] = np.stack([one, zero, zero], -1)
    X[:, 1] = np.stack([b, c, c], -1)
    Y[:, 1] = np.stack([c, c, d], -1)
    Z[:, 1] = np.stack([-d, one, b], -1)
    W[:, 1] = np.stack([zero, one, zero], -1)
    X[:, 2] = np.stack([b, c, d], -1)
    Y[:, 2] = np.stack([d, d, d], -1)
    Z[:, 2] = np.stack([c, -b, one], -1)
    W[:, 2] = np.stack([zero, zero, one], -1)

    b4 = np.stack([one, b, c, d], -1)[:, None, :]  # [n_total, 1, 4]
    pk = np.empty((n_total, N, PLEN), dtype=np.float32)
    pk[:, :, PA:PA + 4] = b4
    pk[:, :, PA + 4:PA + 7] = X
    pk[:, :, PA + 7:PA + 10] = X
    pk[:, :, PBv:PBv + 4] = 0.5 * b4
    pk[:, :, PBv + 4:PBv + 7] = Y
    pk[:, :, PBv + 7:PBv + 10] = -Y
    pk[:, :, PZ:PZ + 3] = Z
    pk[:, :, PW:PW + 3] = W
    pk[:, :, PT] = t_vector
    # replicate over point groups: (n_total, N, PLEN) -> (n_total, N, G, PLEN)
    pk = np.broadcast_to(pk[:, :, None, :], (n_total, N, G, PLEN))
    pk = np.ascontiguousarray(pk).reshape(n_total * N * G, PLEN)

    # coords, planar per partition: xt[(j,n,g), m*Q + q] = pred[j, g*Q+q, m]
    padded = np.zeros((n_total, PADPTS, 3), dtype=np.float32)
    padded[:, :NPTS] = pred_coor
    # (j, g, q, m) -> (j, g, m, q)
    xt = padded.reshape(n_total, G, Q, 3).transpose(0, 1, 3, 2)
    xt = np.broadcast_to(xt[:, None], (n_total, N, G, 3, Q))
    xt = np.ascontiguousarray(xt).reshape(n_total * N * G, 3 * Q)

    jper = J * N * G
    return [
        {
            "pk": np.ascontiguousarray(pk[cc * jper:(cc + 1) * jper]),
            "xt": np.ascontiguousarray(xt[cc * jper:(cc + 1) * jper]),
        }
        for cc in range(NCORES)
    ]


def run(pred_coor, r_vector, t_vector, trace=False):
    from concourse.bass_utils import run_bass_kernel_spmd

    nc = get_nc()
    in_maps = shard_inputs(pred_coor, r_vector, t_vector)
    res = run_bass_kernel_spmd(nc, in_maps, list(range(NCORES)), trace=trace)
    full = np.concatenate(
        [
            np.asarray(res.results[cc]["o3"]).transpose(0, 2, 1)[:, :NPTS, :]
            for cc in range(NCORES)
        ],
        axis=0,
    )
    return np.ascontiguousarray(full), res


def kernel(pred_coor, r_vector, t_vector):
    pred_coor = np.asarray(pred_coor, dtype=np.float32)
    r_vector = np.asarray(r_vector, dtype=np.float32)
    t_vector = np.asarray(t_vector, dtype=np.float32)
    full, _ = run(pred_coor, r_vector, t_vector, trace=False)
    return full


# revision 16
# speedup vs baseline: 1.4411x; 1.4411x over previous
"""Trainium2 Bass kernel for nn_Align: batched quaternion->rotmat + rigid transform.

reference math (per structure j of 64):
    q = (1, b, c, d) / sqrt(s),  s = 1 + b^2 + c^2 + d^2
    R = rotmat(q)                       # 3x3
    out[j] = pred[j] @ R + t[j]         # [91,3] @ [3,3] + [3]

Sharding: data-parallel over the 8 NeuronCores, 8 structures per core.

Per-core layout: partitions = (structure j:8, out-channel n:3, point-group
g:5) = 120, with 19 points per group (91 points padded to 95; the pad is
sliced off host-side).  Every rotation-matrix entry has the form
    R[k,n] = (x*y + z) * (2/s) - w
with (x,y,z,w) in {1,b,c,d,0,+-...}, so column n of R is computed per
partition from host-packed operand vectors X,Y,Z,W via
    C = (X*Y + Z) * (2/s) - W            # C = (R[0,n], R[1,n], R[2,n])
and s comes from one fused tensor_tensor_reduce (products + scaled sum).
The transform is then only 3 multiply-add ops over the 19-point free dim:
    out_n = ((x0*C0 + t_n) + x1*C1) + x2*C2
Output is written channel-planar ([3,8,95] DRAM) so each partition's row is
contiguous; the host interleaves channels during the unshard gather.

Two parallel input DMAs: the small params tensor goes through the SP queue
(fast DGE path -- it gates the compute chain) while the 3x-replicated
coordinate tensor rides the otherwise-idle Activation-engine queue.

Raw Bass (no Tile: this walrus build encodes at most one sync-wait per
compute instruction).  Every DVE RAW dep is semaphore-synced (streaming
same-engine RAW is not safe on HW), and the kernel clears its semaphores
then all-engine-barriers before use (sems persist across NEFF runs).
"""

import numpy as np

NCORES = 8
J = 8          # structures per core
N = 3          # output channels (partition dim)
G = 5          # point groups per (structure, channel)
Q = 19         # points per group (G*Q = 95 >= 91)
NPTS = 91
PADPTS = G * Q  # 95
PARTS = J * N * G  # 120 partitions

# params row: [A(10): 1 b c d x0 x1 x2 x0 x1 x2 |
#              B(10): .5 .5b .5c .5d y0 y1 y2 -y0 -y1 -y2 |
#              Z(3) | W(3) | t(1)] = 27 floats
# One fused (A*B, accum) op then yields the column products AND s/2: the
# +-x*y accumulator slots cancel, leaving 0.5*(1+b^2+c^2+d^2).
PA, PBv, PZ, PW, PT, PLEN = 0, 10, 20, 23, 26, 27

_cache = {}


def _build_nc():
    import concourse.bass as bass
    import concourse.mybir as mybir

    f32 = mybir.dt.float32
    Alu = mybir.AluOpType

    nc = bass.Bass()
    pk_d = nc.dram_tensor("pk", [PARTS, PLEN], f32, kind="ExternalInput")
    xt_d = nc.dram_tensor("xt", [PARTS, 3 * Q], f32, kind="ExternalInput")
    # channel-planar output: [j, n, 95]; host transposes/slices to [j,91,n]
    o3 = nc.dram_tensor("o3", [J, N, PADPTS], f32, kind="ExternalOutput")

    with (
        nc.sbuf_tensor([PARTS, PLEN], f32) as PK_t,
        nc.sbuf_tensor([PARTS, 3 * Q], f32) as XT_t,
        nc.sbuf_tensor([PARTS, 10], f32) as PR_t,
        nc.sbuf_tensor([PARTS, 1], f32) as S2_t,
        nc.sbuf_tensor([PARTS, 1], f32) as INV_t,
        nc.sbuf_tensor([PARTS, 3], f32) as NU_t,
        nc.sbuf_tensor([PARTS, 3], f32) as C_t,
        nc.sbuf_tensor([PARTS, Q], f32) as A1_t,
        nc.sbuf_tensor([PARTS, Q], f32) as A2_t,
        nc.sbuf_tensor([PARTS, Q], f32) as O_t,
        nc.semaphore("p_sem") as p_sem,
        nc.semaphore("x_sem") as x_sem,
        nc.semaphore("v_sem") as v_sem,
        nc.semaphore("o_sem") as o_sem,
        nc.Block() as block,
    ):
        PK = PK_t[:, :]
        XT = XT_t[:, :]
        A1 = A1_t[:, :]
        A2 = A2_t[:, :]
        O = O_t[:, :]

        def _pseudo_barrier(eng):
            # NRT expands this to a per-engine DGE drain + all-engine
            # barrier on runtime semaphores outside the kernel sem range --
            # stale-state proof, and the drain retires each engine's
            # outstanding DMA state.
            eng.isa(
                nc.isa.Opcode.NEURON_ISA_TPB_OPCODE_PSEUDO_SYNC_BARRIER,
                {},
                struct_name="NEURON_ISA_TPB_UNKNOWN_STRUCT",
                verify=False,
            )

        @block.gpsimd
        def _(gpsimd):
            # Stale-semaphore preamble: semaphores are NOT reset between NEFF
            # executions, and waits here use absolute values.  Clear every
            # sem this kernel waits on or increments, THEN barrier -- without
            # the barrier an engine can pass its first wait on a stale value
            # before the clear lands (observed as a HW deadlock).
            #
            # The two input DMAs are issued by SP/Act BEFORE this preamble
            # completes, so p_sem/x_sem are excluded from the dma_reset
            # (draining them would nuke the in-flight descriptors; their
            # queues are safe because the previous run of this kernel fully
            # retired both input DMAs before its exit barrier).  The
            # clear-vs-inc race is safe by construction: a HWDGE completion
            # inc lands >=1.5us after issue (descriptor gen + DGE delay +
            # transfer + sem prop), while these clears retire within ~400ns
            # of kernel start; a pathologically late clear would zero the
            # sem after its inc and the consumer wait would hang (fail-stop,
            # not silent corruption).  All consumer waits still execute
            # after the barrier, hence after the clears.
            nums = sorted(x.num for x in (p_sem, x_sem, v_sem, o_sem))
            assert nums[-1] - nums[0] == 3, nums
            assert v_sem.num + 1 == o_sem.num and x_sem.num < v_sem.num
            gpsimd.dma_reset(range(v_sem.num, o_sem.num + 1))
            gpsimd.sem_clear(range(nums[0], nums[-1] + 1))
            _pseudo_barrier(gpsimd)

        @block.tensor
        def _(tensor):
            _pseudo_barrier(tensor)

        @block.scalar
        def _(scalar):
            # coordinate tensor rides the Activation HWDGE queue, parallel
            # with the params DMA on SP; issued before the preamble barrier
            # (see gpsimd block for why that is safe)
            scalar.dma_start(out=XT, in_=xt_d[:, :]).then_inc(x_sem, 16)
            _pseudo_barrier(scalar)

        @block.sync
        def _(sync):
            sync.dma_start(out=PK, in_=pk_d[:, :]).then_inc(p_sem, 16)
            _pseudo_barrier(sync)
            sync.wait_ge(v_sem, 7)
            sync.dma_start(
                out=o3[:, :, :].rearrange("j n (g q) -> (j n g) q", g=G),
                in_=O,
            ).then_inc(o_sem, 16)
            sync.wait_ge(o_sem, 16)

        @block.vector
        def _(vector):
            _pseudo_barrier(vector)
            vector.wait_ge(p_sem, 16)

            # DVE streaming RAW is not safe without sem sync (HW-verified):
            # every op bumps v_sem; consumers wait on the cumulative count.
            def op(k, *args, **kw):
                return getattr(vector, k)(*args, **kw).then_inc(v_sem, 1)

            # ---- rotation column C = (X*Y + Z) * (2/s) - W ----
            # one fused op: PR = A*B elementwise (PR[4:7] = x*y column
            # products) and accum = sum(PR) = s/2 (the +-x*y slots cancel).
            # (tensor_tensor_reduce hits "ISA wrong length" in neuronxcc
            # codegen; scalar_tensor_tensor's accum_out compiles fine)
            op("scalar_tensor_tensor", out=PR_t[:, :],                   # 1
               in0=PK[:, PA:PA + 10], scalar=1.0, in1=PK[:, PBv:PBv + 10],
               op0=Alu.mult, op1=Alu.mult, accum_out=S2_t[:, :])
            vector.wait_ge(v_sem, 1)
            op("reciprocal", out=INV_t[:, :], in_=S2_t[:, :])            # 2  2/s
            op("tensor_tensor", out=NU_t[:, :], in0=PR_t[:, 4:7],        # 3
               in1=PK[:, PZ:PZ + 3], op=Alu.add)
            vector.wait_ge(v_sem, 3)
            op("scalar_tensor_tensor", out=C_t[:, :], in0=NU_t[:, :],    # 4
               scalar=INV_t[:, 0:1], in1=PK[:, PW:PW + 3],
               op0=Alu.mult, op1=Alu.subtract)

            # ---- transform: out_n = ((x0*C0 + t) + x1*C1) + x2*C2 ----
            vector.wait_ge(x_sem, 16)
            vector.wait_ge(v_sem, 4)
            op("tensor_scalar", out=A1, in0=XT[:, 0:Q],                  # 5
               scalar1=C_t[:, 0:1], scalar2=PK[:, PT:PT + 1],
               op0=Alu.mult, op1=Alu.add)
            vector.wait_ge(v_sem, 5)
            op("scalar_tensor_tensor", out=A2, in0=XT[:, Q:2 * Q],       # 6
               scalar=C_t[:, 1:2], in1=A1, op0=Alu.mult, op1=Alu.add)
            vector.wait_ge(v_sem, 6)
            op("scalar_tensor_tensor", out=O, in0=XT[:, 2 * Q:3 * Q],    # 7
               scalar=C_t[:, 2:3], in1=A2, op0=Alu.mult, op1=Alu.add)

    return nc


def get_nc():
    if "nc" not in _cache:
        _cache["nc"] = _build_nc()
    return _cache["nc"]


def shard_inputs(pred_coor, r_vector, t_vector):
    n_total = pred_coor.shape[0]
    b, c, d = r_vector[:, 0], r_vector[:, 1], r_vector[:, 2]
    one = np.ones_like(b)
    zero = np.zeros_like(b)

    # per-channel operand vectors: R[k,n] = (x*y+z)*(2/s) - w
    X = np.empty((n_total, N, 3), dtype=np.float32)
    Y = np.empty((n_total, N, 3), dtype=np.float32)
    Z = np.empty((n_total, N, 3), dtype=np.float32)
    W = np.empty((n_total, N, 3), dtype=np.float32)
    X[:, 0] = np.stack([b, b, b], -1)
    Y[:, 0] = np.stack([b, c, d], -1)
    Z[:, 0] = np.stack([one, d, -c], -1)
    W[:, 0] = np.stack([one, zero, zero], -1)
    X[:, 1] = np.stack([b, c, c], -1)
    Y[:, 1] = np.stack([c, c, d], -1)
    Z[:, 1] = np.stack([-d, one, b], -1)
    W[:, 1] = np.stack([zero, one, zero], -1)
    X[:, 2] = np.stack([b, c, d], -1)
    Y[:, 2] = np.stack([d, d, d], -1)
    Z[:, 2] = np.stack([c, -b, one], -1)
    W[:, 2] = np.stack([zero, zero, one], -1)

    b4 = np.stack([one, b, c, d], -1)[:, None, :]  # [n_total, 1, 4]
    pk = np.empty((n_total, N, PLEN), dtype=np.float32)
    pk[:, :, PA:PA + 4] = b4
    pk[:, :, PA + 4:PA + 7] = X
    pk[:, :, PA + 7:PA + 10] = X
    pk[:, :, PBv:PBv + 4] = 0.5 * b4
    pk[:, :, PBv + 4:PBv + 7] = Y
    pk[:, :, PBv + 7:PBv + 10] = -Y
    pk[:, :, PZ:PZ + 3] = Z
    pk[:, :, PW:PW + 3] = W
    pk[:, :, PT] = t_vector
    # replicate over point groups: (n_total, N, PLEN) -> (n_total, N, G, PLEN)
    pk = np.broadcast_to(pk[:, :, None, :], (n_total, N, G, PLEN))
    pk = np.ascontiguousarray(pk).reshape(n_total * N * G, PLEN)

    # coords, planar per partition: xt[(j,n,g), m*Q + q] = pred[j, g*Q+q, m]
    padded = np.zeros((n_total, PADPTS, 3), dtype=np.float32)
    padded[:, :NPTS] = pred_coor
    # (j, g, q, m) -> (j, g, m, q)
    xt = padded.reshape(n_total, G, Q, 3).transpose(0, 1, 3, 2)
    xt = np.broadcast_to(xt[:, None], (n_total, N, G, 3, Q))
    xt = np.ascontiguousarray(xt).reshape(n_total * N * G, 3 * Q)

    jper = J * N * G
    return [
        {
            "pk": np.ascontiguousarray(pk[cc * jper:(cc + 1) * jper]),
            "xt": np.ascontiguousarray(xt[cc * jper:(cc + 1) * jper]),
        }
        for cc in range(NCORES)
    ]


def run(pred_coor, r_vector, t_vector, trace=False):
    from concourse.bass_utils import run_bass_kernel_spmd

    nc = get_nc()
    in_maps = shard_inputs(pred_coor, r_vector, t_vector)
    res = run_bass_kernel_spmd(nc, in_maps, list(range(NCORES)), trace=trace)
    full = np.concatenate(
        [
            np.asarray(res.results[cc]["o3"]).transpose(0, 2, 1)[:, :NPTS, :]
            for cc in range(NCORES)
        ],
        axis=0,
    )
    return np.ascontiguousarray(full), res


def kernel(pred_coor, r_vector, t_vector):
    pred_coor = np.asarray(pred_coor, dtype=np.float32)
    r_vector = np.asarray(r_vector, dtype=np.float32)
    t_vector = np.asarray(t_vector, dtype=np.float32)
    full, _ = run(pred_coor, r_vector, t_vector, trace=False)
    return full
